# revision 6
# baseline (speedup 1.0000x reference)
"""MinGRU (2-layer, residual) Trainium2 Bass kernel.

Problem: B=8, S=2048, D=H=1024, L=2.
Sharding: data-parallel over batch across 8 NeuronCores (1 sample/core);
weights replicated. All tensors device-side are channel-major (channels on
partitions, sequence on the free dimension) so no transposes are needed on
device; the host transposes x / W once and transposes the output back.

Per-core pipeline (per layer):
  gh^T = W^T-blocks @ x^T            TensorE, fp32r (full rate, ~1e-4 rel err)
  c = sigmoid(-(gate+b))             ScalarE from PSUM, fused bias/scale
  z = sigmoid(gate+b)                ScalarE
  g = max(hidden+b+0.5, sigmoid(hidden+b))   ScalarE + fused DVE scalar_tensor_tensor
  u = z*g                            DVE
  h_t = c_t*h_{t-1} + u_t            DVE tensor_tensor_scan along S
  residual adds                      GPSIMD
Layer 0 streams x chunk-by-chunk (t-chunks of 512); layer 1 reuses the
resident inp = h+x tiles as matmul rhs.
"""
import numpy as np

import concourse.bass as bass
import concourse.mybir as mybir
import concourse.tile as tile
from concourse import bacc
from concourse.bass_utils import run_bass_kernel_spmd

F32 = mybir.dt.float32
F32R = mybir.dt.float32r
SIG = mybir.ActivationFunctionType.Sigmoid
MULT = mybir.AluOpType.mult
ADD = mybir.AluOpType.add
MAX = mybir.AluOpType.max

B, S, D, H = 8, 2048, 1024, 1024
KT = D // 128          # 8  k-tiles (contraction)
HT = H // 128          # 8  h-tiles (per gate/hidden half)
ET = 2 * H // 128      # 16 e-tiles (2H output channels)
TC = 4                 # t-chunks
CS = S // TC           # 512 chunk size (PSUM bank width in fp32)

_CACHED = {}


def build():
    nc = bacc.Bacc()

    xt = nc.dram_tensor("xt", [KT, 128, S], F32R, kind="ExternalInput")
    w0t = nc.dram_tensor("w0t", [KT, 128, 2 * H], F32R, kind="ExternalInput")
    w1t = nc.dram_tensor("w1t", [KT, 128, 2 * H], F32R, kind="ExternalInput")
    bias = nc.dram_tensor("bias", [128, 2 * ET], F32, kind="ExternalInput")
    aux = nc.dram_tensor("aux", [128, 2 * ET], F32, kind="ExternalInput")
    hinit = nc.dram_tensor("hinit", [128, 2 * HT], F32, kind="ExternalInput")

    outT = nc.dram_tensor("outT", [HT, 128, S], F32, kind="ExternalOutput")
    hfin = nc.dram_tensor("hfin", [2, HT, 128], F32, kind="ExternalOutput")

    with tile.TileContext(nc) as tc:
        with (
            tc.tile_pool(name="singles", bufs=1) as singles,
            tc.tile_pool(name="wpool", bufs=1) as wpool,
            tc.tile_pool(name="chunks", bufs=2) as chunks,
            tc.tile_pool(name="cz", bufs=1) as czpool,
            tc.tile_pool(name="inp", bufs=1) as inppool,
            tc.tile_pool(name="psum", bufs=6, space="PSUM") as psum_pool,
        ):
            bias_sb = singles.tile([128, 2 * ET], F32)
            aux_sb = singles.tile([128, 2 * ET], F32)
            hinit_sb = singles.tile([128, 2 * HT], F32)
            hst1 = singles.tile([128, HT], F32)
            nc.sync.dma_start(out=bias_sb, in_=bias.ap())
            nc.sync.dma_start(out=aux_sb, in_=aux.ap())
            nc.sync.dma_start(out=hinit_sb, in_=hinit.ap())

            inp_sb = inppool.tile([128, KT, S], F32R, tag="inp")

            def layer(lidx):
                """Emit one MinGRU layer. lidx 0: rhs = x (streamed);
                lidx 1: rhs = inp_sb (resident)."""
                bofs = lidx * ET      # bias/aux column offset
                hofs = lidx * HT      # hinit column offset
                wdram = w0t if lidx == 0 else w1t

                w_sb = wpool.tile([128, KT, 2 * H], F32R, tag="w")
                for k in range(KT):
                    nc.sync.dma_start(out=w_sb[:, k, :], in_=wdram.ap()[k])

                prev_x = None
                for t in range(TC):
                    ts0, ts1 = t * CS, (t + 1) * CS
                    if lidx == 0:
                        x_t = chunks.tile([128, KT, CS], F32R, tag="chunkbuf")
                        for k in range(KT):
                            nc.sync.dma_start(
                                out=x_t[:, k, :], in_=xt.ap()[k, :, ts0:ts1]
                            )
                        rhs = lambda k: x_t[:, k, :]
                    else:
                        h1_t = chunks.tile([128, KT, CS], F32, tag="chunkbuf")
                        rhs = lambda k: inp_sb[:, k, ts0:ts1]

                    c_t = czpool.tile([128, HT, CS], F32, tag="c")
                    gu_t = czpool.tile([128, HT, CS], F32, tag="gu")

                    for i in range(HT):
                        # ---- gate e-tile i ----
                        ps = psum_pool.tile([128, CS], F32, tag="ps")
                        for k in range(KT):
                            nc.tensor.matmul(
                                ps,
                                lhsT=w_sb[:, k, i * 128:(i + 1) * 128],
                                rhs=rhs(k),
                                start=(k == 0),
                                stop=(k == KT - 1),
                            )
                        nc.scalar.activation(
                            out=c_t[:, i, :], in_=ps, func=SIG,
                            bias=aux_sb[:, bofs + i:bofs + i + 1], scale=-1.0,
                        )
                        # z = sigmoid(gate+b), written in place into the gate
                        # PSUM bank (saves an SBUF chunk buffer); consumed by
                        # the u-mult below.
                        nc.scalar.activation(
                            out=ps, in_=ps, func=SIG,
                            bias=bias_sb[:, bofs + i:bofs + i + 1],
                        )
                        # ---- hidden e-tile i+HT ----
                        ph = psum_pool.tile([128, CS], F32, tag="ps")
                        e = HT + i
                        for k in range(KT):
                            nc.tensor.matmul(
                                ph,
                                lhsT=w_sb[:, k, e * 128:(e + 1) * 128],
                                rhs=rhs(k),
                                start=(k == 0),
                                stop=(k == KT - 1),
                            )
                        nc.scalar.activation(
                            out=gu_t[:, i, :], in_=ph, func=SIG,
                            bias=bias_sb[:, bofs + e:bofs + e + 1],
                        )
                        # g = (hidden + (b+0.5)) max sigmoid(hidden+b)
                        nc.vector.scalar_tensor_tensor(
                            out=gu_t[:, i, :], in0=ph,
                            scalar=aux_sb[:, bofs + e:bofs + e + 1],
                            in1=gu_t[:, i, :], op0=ADD, op1=MAX,
                        )
                        # u = z * g (z still lives in the gate PSUM bank)
                        nc.vector.tensor_mul(gu_t[:, i, :], ps, gu_t[:, i, :])
                        # ---- scan ----
                        if lidx == 0:
                            init = (hinit_sb[:, hofs + i:hofs + i + 1] if t == 0
                                    else inp_sb[:, i, ts0 - 1:ts0])
                            nc.vector.tensor_tensor_scan(
                                out=inp_sb[:, i, ts0:ts1],
                                data0=c_t[:, i, :], data1=gu_t[:, i, :],
                                initial=init, op0=MULT, op1=ADD,
                            )
                        else:
                            init = (hinit_sb[:, hofs + i:hofs + i + 1] if t == 0
                                    else hst1[:, i:i + 1])
                            nc.vector.tensor_tensor_scan(
                                out=h1_t[:, i, :],
                                data0=c_t[:, i, :], data1=gu_t[:, i, :],
                                initial=init, op0=MULT, op1=ADD,
                            )

                    if lidx == 0:
                        # lazy residual add: inp[t-1] += x[t-1] — emitted after
                        # chunk t's scans so the scan chain reads pre-residual h.
                        if t > 0:
                            nc.gpsimd.tensor_add(
                                inp_sb[:, :, ts0 - CS:ts0],
                                inp_sb[:, :, ts0 - CS:ts0],
                                prev_x,
                            )
                        prev_x = x_t
                    else:
                        # carry hst1 for next chunk's scan initial
                        for i in range(HT):
                            nc.gpsimd.tensor_copy(
                                hst1[:, i:i + 1], h1_t[:, i, CS - 1:CS]
                            )
                        if t == TC - 1:
                            for i in range(HT):
                                nc.sync.dma_start(
                                    out=hfin.ap()[1, i].unsqueeze(1),
                                    in_=h1_t[:, i, CS - 1:CS],
                                )
                        # out = h1 + inp (in place), then DMA out
                        nc.gpsimd.tensor_add(
                            h1_t, h1_t, inp_sb[:, :, ts0:ts1].bitcast(F32)
                        )
                        for i in range(HT):
                            nc.sync.dma_start(
                                out=outT.ap()[i, :, ts0:ts1], in_=h1_t[:, i, :]
                            )

                if lidx == 0:
                    # h0_final = pre-residual h at s = S-1 (before the lazy add)
                    for i in range(HT):
                        nc.sync.dma_start(
                            out=hfin.ap()[0, i].unsqueeze(1),
                            in_=inp_sb[:, i, S - 1:S].bitcast(F32),
                        )
                    # final lazy residual add for the last chunk
                    nc.gpsimd.tensor_add(
                        inp_sb[:, :, S - CS:S],
                        inp_sb[:, :, S - CS:S],
                        prev_x,
                    )

            layer(0)
            layer(1)

    nc.compile()
    return nc


def _prepare_shared(w0, b0, w1, b1, h):
    w0t = np.ascontiguousarray(w0.T).reshape(KT, 128, 2 * H)
    w1t = np.ascontiguousarray(w1.T).reshape(KT, 128, 2 * H)
    # bias[:, l*16 + e] = b_l[e*128 : (e+1)*128]
    bias = np.concatenate(
        [b0.reshape(ET, 128).T, b1.reshape(ET, 128).T], axis=1
    ).astype(np.float32)
    aux0 = np.concatenate([-b0[:H].reshape(HT, 128).T,
                           b0[H:].reshape(HT, 128).T + 0.5], axis=1)
    aux1 = np.concatenate([-b1[:H].reshape(HT, 128).T,
                           b1[H:].reshape(HT, 128).T + 0.5], axis=1)
    aux = np.concatenate([aux0, aux1], axis=1).astype(np.float32)
    return w0t, w1t, bias, np.ascontiguousarray(aux)


def kernel(x, h, w0, b0, w1, b1):
    x = np.asarray(x, np.float32)
    h = np.asarray(h, np.float32)
    w0 = np.asarray(w0, np.float32)
    b0 = np.asarray(b0, np.float32)
    w1 = np.asarray(w1, np.float32)
    b1 = np.asarray(b1, np.float32)

    if "nc" not in _CACHED:
        _CACHED["nc"] = build()
    nc = _CACHED["nc"]

    w0t, w1t, bias, aux = _prepare_shared(w0, b0, w1, b1, h)
    in_maps = []
    for b in range(B):
        xt = np.ascontiguousarray(x[b].T).reshape(KT, 128, S)
        hinit = np.concatenate(
            [h[0, b, 0].reshape(HT, 128).T, h[1, b, 0].reshape(HT, 128).T],
            axis=1,
        ).astype(np.float32)
        in_maps.append({
            "xt": xt, "w0t": w0t, "w1t": w1t,
            "bias": bias, "aux": aux,
            "hinit": np.ascontiguousarray(hinit),
        })

    res = run_bass_kernel_spmd(nc, in_maps, core_ids=list(range(B)))

    out = np.empty((B, S, H), np.float32)
    hfinal = np.empty((2, B, 1, H), np.float32)
    for b in range(B):
        r = res.results[b]
        out[b] = r["outT"].reshape(H, S).T
        hfinal[:, b, 0, :] = r["hfin"].reshape(2, H)
    return out, hfinal
